# revision 40
# baseline (speedup 1.0000x reference)
"""Trainium2 Bass kernel for nn_ColorGrid (bilinear grid_sample of two
[3,400,400] tables at 8x524288 points, sigmoid on the color channels).

Strategy (data-parallel over 8 NeuronCores, one batch row each):

  A naive kernel gathers one 256B brick per point with bulk dma_gather;
  at the cost-model floor of 22.76ns per 256B descriptor / 16 DMA
  engines that serializes to ~745us/core on the DMA engines.  Points
  are uniform, so on average ~13 points land in the same 3x3-cell
  brick: host-side grouping lets ONE descriptor serve G=6 points.

  1. Brick table P3 (built on host, f16): slot (a,b), a,b in [0,200],
     holds the 3x3 cell neighborhood rows 2a-1..2a+1, cols 2b-1..2b+1
     of both tables (zeros off-table), laid out [h:6][t:9] (t = 3*or+oc)
     in a 256B slot. A point with cell (y0,x0) needs the 2x2 corner
     window inside slot (a,b) = ((y0+1)//2, (x0+1)//2).
  2. Host computes per-point bilinear weights, scatters them into a
     9-tap f16 vector wxy aligned with the brick layout (4 nonzeros),
     sorts points by slot id and packs G=6 points per gather descriptor
     (padding partial groups with zero-weight dummies). Slot ids are
     binned in two row-ranges so bin-relative ids fit in int16; gather
     instructions for bin-1 tiles use a base-offset view of P3.
  3. Device per tile (C=126 point cols, M=21 groups/partition-row;
     gathers issued per tile-pair to halve SWDGE overhead, except the
     first two tiles which gather singly so compute starts sooner):
     load wxy + prefolded idx, dma_gather, multiply the gathered block
     (broadcast 0-stride over the G dim) by wxy on DVE, reduce the 9
     taps with an add tree split across DVE and the Pool engine,
     sigmoid the color channels in-place on ACT, store [128, C*6] f16.
     Engine balance per tile: DVE ~6.0us (products + tree top), Pool
     ~6.0us (descriptor gen + tree bottom), DMA ~5.3us, ACT ~0.5us.
  4. Host scatters the padded device output back to original point
     order and casts f32.
"""

import numpy as np

import concourse.bacc as bacc
import concourse.mybir as mybir
import concourse.tile as tile
from concourse import library_config
from concourse.bass_utils import run_bass_kernel_spmd
from concourse.tile_rust import add_dep_helper

F32 = mybir.dt.float32
F16 = mybir.dt.float16
I16 = mybir.dt.int16
Alu = mybir.AluOpType
Sigmoid = mybir.ActivationFunctionType.Sigmoid

P = 128            # SBUF partitions
W = 400            # table size
NA = 201           # brick rows (a = (y0+1)//2, y0 in [-1,399])
NB = 201           # brick cols
S_TOT = NA * NB    # 40401 slots
A_SPLIT = 99       # bin0 split chosen so both bins fill their tiles
BASE1 = A_SPLIT * NB          # 20301 (slot-id offset of bin 1)
ELEM = 128         # f16 per slot (54 used)

G = 6              # points per gather descriptor (grouped by slot)
C = 174            # point cols per tile (must be divisible by G)
M = C // G         # gather blocks per partition-row
NT = P * C         # points per tile
NI = P * M         # gather idxs per tile
C_DVE = 102        # s2 column split: [0,C_DVE) on DVE, rest on Pool


def unit_list(t0, t1):
    """Gather units (tbase, ntile): tiles 0,1 single for fast start,
    then pairs within each bin, leftover bin tiles as singles."""
    assert t0 >= 2
    units = [(0, 1), (1, 1)]
    for lo, hi in ((2, t0), (t0, t0 + t1)):
        t = lo
        while t + 1 < hi:
            units.append((t, 2))
            t += 2
        if t < hi:
            units.append((t, 1))
    return units

N_CORES = 8
N_FULL = 524288


def build_nc(t0, t1, n_cores=N_CORES):
    """Compile the SPMD program: t0 bin-0 tiles followed by t1 bin-1 tiles.

    Gathers and input loads are issued per gather unit (see unit_list):
    mostly tile-pairs, halving the fixed SWDGE descriptor-generation
    overhead on Pool; single-tile units for fast pipeline fill and odd
    bin leftovers.
    """
    tt = t0 + t1
    units = unit_list(t0, t1)
    nc = bacc.Bacc(
        "TRN2", target_bir_lowering=False, debug=False, num_devices=n_cores
    )
    p3_d = nc.dram_tensor("p3", [S_TOT * ELEM], F16, kind="ExternalInput")
    w_d = nc.dram_tensor("wxy", [tt * NT * 9], F16, kind="ExternalInput")
    icd_d = nc.dram_tensor("icd", [len(units) * P * 16 * M], I16,
                           kind="ExternalInput")
    out_d = nc.dram_tensor("out", [tt * NT * 6], F16, kind="ExternalOutput")

    nu = len(units)
    p3_slots = p3_d.ap().rearrange("(s e) -> s e", e=ELEM)
    w_rows = w_d.ap().rearrange("(t p c n) -> t p (c n)", t=tt, p=P, n=9)
    icd_rows = icd_d.ap().rearrange("(u p m) -> u p m", u=nu, p=P)
    out_rows = out_d.ap().rearrange("(t p c h) -> t p (c h)", t=tt, p=P, h=6)

    with tile.TileContext(nc) as tc:
        lib = nc.gpsimd.load_library(library_config.mlp)

        with tc.tile_pool(name="inp", bufs=3) as pool_i, tc.tile_pool(
            name="work", bufs=4
        ) as pool_w, tc.tile_pool(name="outp", bufs=3) as pool_o:
            staged = {}

            def emit_gather(u):
                tbase, ntile = units[u]
                idx = pool_i.tile([P, 16 * M], I16, tag="idx")
                nc.sync.dma_start(
                    out=idx[:, : ntile * 8 * M], in_=icd_rows[u][:, : ntile * 8 * M]
                )
                g = pool_i.tile([P, 2 * M * ELEM], F16, tag="g")
                gv = g[:].rearrange("p (m e) -> p m e", e=ELEM)
                src = p3_slots if tbase < t0 else p3_slots[BASE1:]
                gi = nc.gpsimd.dma_gather(
                    gv[:, : ntile * M, :], src, idx[:, : ntile * 8 * M],
                    ntile * NI, ntile * NI, ELEM,
                    single_packet=False,
                )
                add_dep_helper(gi.ins, lib.ins, reason="gather needs mlp lib")
                wxy = pool_i.tile([P, 2 * C * 9], F16, tag="wxy")
                nc.sync.dma_start(
                    out=wxy[:, : ntile * C * 9],
                    in_=w_rows[tbase : tbase + ntile].transpose([1, 0, 2]),
                )
                for k in range(ntile):
                    staged[tbase + k] = (g, wxy, k)

            def emit_blend(t):
                last = t == tt - 1
                ve = nc.vector if last else nc.gpsimd
                cd = C if last else C_DVE
                g, wxy2, k = staged.pop(t)
                gv = g[:].rearrange("p (m e) -> p m e", e=ELEM)[
                    :, k * M : (k + 1) * M, :
                ]
                wv = wxy2[:].rearrange(
                    "p (k m g n) -> p k m g n", k=2, m=M, n=9
                )[:, k]

                # prod[p, c, h, t'] = g[p, c//G, 9h+t'] * wxy[p, c, t']
                pr = pool_w.tile([P, C * 54], F16, tag="pr")
                prv = pr[:].rearrange("p (c h n) -> p c h n", h=6, n=9)
                for h in range(6):
                    nc.vector.tensor_tensor(
                        prv[:, :, h, :].rearrange("p (m g) n -> p m g n", g=G),
                        gv[:, :, 9 * h : 9 * h + 9]
                        .unsqueeze(2)
                        .broadcast_to([P, M, G, 9]),
                        wv,
                        Alu.mult,
                    )
                # reduce the 9 taps: s1/s2 on DVE (s2 column-split with
                # Pool), s3 + tap-8 add on Pool
                s1 = pool_w.tile([P, C * 24], F16, tag="s1")
                s1v = s1[:].rearrange("p (c h n) -> p c h n", h=6, n=4)
                nc.vector.tensor_tensor(
                    s1v, prv[:, :, :, 0:4], prv[:, :, :, 4:8], Alu.add
                )
                s2 = pool_w.tile([P, C * 12], F16, tag="s2")
                s2v = s2[:].rearrange("p (c h n) -> p c h n", h=6, n=2)
                nc.vector.tensor_tensor(
                    s2v[:, :cd],
                    s1v[:, :cd, :, 0:2],
                    s1v[:, :cd, :, 2:4],
                    Alu.add,
                )
                if cd < C:
                    nc.gpsimd.tensor_tensor(
                        s2v[:, cd:],
                        s1v[:, cd:, :, 0:2],
                        s1v[:, cd:, :, 2:4],
                        Alu.add,
                    )
                s3 = pool_w.tile([P, C * 6], F16, tag="s3")
                s3v = s3[:].rearrange("p (c h) -> p c h", h=6)
                ve.tensor_tensor(
                    s3v, s2v[:, :, :, 0], s2v[:, :, :, 1], Alu.add
                )
                o6 = pool_o.tile([P, C * 6], F16, tag="o6")
                o6v = o6[:].rearrange("p (c h) -> p c h", h=6)
                ve.tensor_tensor(o6v, s3v, prv[:, :, :, 8], Alu.add)
                # sigmoid color channels in place, grid channels pass through
                nc.scalar.activation(o6v[:, :, 0:3], o6v[:, :, 0:3], Sigmoid)
                nc.scalar.dma_start(out=out_rows[t], in_=o6[:])

            tile_unit = {}
            for ui, (tb, nt_) in enumerate(units):
                for kk in range(nt_):
                    tile_unit[tb + kk] = ui
            emit_gather(0)
            emit_gather(1)
            emitted = 2
            for t in range(tt):
                # keep two units of lookahead beyond the one being consumed
                while emitted < min(tile_unit[t] + 3, nu):
                    emit_gather(emitted)
                    emitted += 1
                emit_blend(t)

    nc.compile()
    return nc


_NC_CACHE = {}


def _get_nc(t0, t1):
    key = (t0, t1)
    if key not in _NC_CACHE:
        _NC_CACHE[key] = build_nc(t0, t1)
    return _NC_CACHE[key]


def _build_p3(color, grid):
    """[S_TOT, ELEM] f16 brick table; slot (a,b) = [h:6][or:3][oc:3]."""
    tpad = np.zeros((6, W + 3, W + 3), np.float32)
    tpad[0:3, 1 : W + 1, 1 : W + 1] = color
    tpad[3:6, 1 : W + 1, 1 : W + 1] = grid
    p3 = np.zeros((NA, NB, ELEM), np.float16)
    for orr in range(3):
        for oc in range(3):
            v = tpad[:, orr : orr + 2 * NA : 2, oc : oc + 2 * NB : 2]
            for h in range(6):
                p3[:, :, 9 * h + 3 * orr + oc] = v[h].astype(np.float16)
    return p3.reshape(-1)


def _prep_core(x):
    """Per-core host prep.

    Returns (d0, d1, point_data) where point_data carries everything
    needed to build the device arrays once common tile counts are known.
    """
    n = x.shape[0]
    cx = x * np.float32(2.0) - np.float32(1.0)
    pos = ((cx + np.float32(1.0)) * np.float32(W) - np.float32(1.0)) * np.float32(0.5)
    f0 = np.floor(pos)
    w1 = pos - f0                       # [n, 2] f32: (wx1, wy1)
    k = f0.astype(np.int64) + 1         # [n, 2]: (kx, ky) in [0, 400]
    bcol = k[:, 0] >> 1
    oc0 = k[:, 0] & 1
    arow = k[:, 1] >> 1
    or0 = k[:, 1] & 1
    slot = arow * NB + bcol             # [n] int64 in [0, S_TOT)

    wx1 = w1[:, 0]
    wy1 = w1[:, 1]
    wx0 = np.float32(1.0) - wx1
    wy0 = np.float32(1.0) - wy1
    w4 = np.stack([wy0 * wx0, wy0 * wx1, wy1 * wx0, wy1 * wx1], axis=1)
    t00 = (or0 * 3 + oc0).astype(np.int64)
    wxy = np.zeros((n, 9), np.float16)
    cols = t00[:, None] + np.array([0, 1, 3, 4], np.int64)[None, :]
    np.put_along_axis(wxy, cols, w4.astype(np.float16), axis=1)

    counts = np.bincount(slot, minlength=S_TOT)
    ngrp = (counts + (G - 1)) // G
    gbase = np.concatenate([[0], np.cumsum(ngrp)[:-1]])
    starts = np.concatenate([[0], np.cumsum(counts)[:-1]])
    order = np.argsort(slot, kind="stable")
    rank = np.empty(n, np.int64)
    rank[order] = np.arange(n) - starts[slot[order]]

    d0 = int(ngrp[:BASE1].sum())
    d1 = int(ngrp[BASE1:].sum())
    return d0, d1, (slot, rank, gbase, ngrp, wxy, d0)


def _build_arrays(point_data, t0, t1):
    """Device arrays for one core at common tile counts (t0, t1)."""
    slot, rank, gbase, ngrp, wxy, d0 = point_data
    tt = t0 + t1
    n = slot.shape[0]

    # group sequence position: bin0 groups at [0, d0); bin1 at t0*NI + ...
    seqbase = gbase.copy()
    seqbase[BASE1:] += t0 * NI - d0
    seq = seqbase[slot] + rank // G
    j = rank % G
    ti = seq // NI
    i = seq % NI
    q = (ti * P + (i % P)) * C + (i // P) * G + j   # padded flat position

    wxy_flat = np.zeros((tt * NT, 9), np.float16)
    wxy_flat[q] = wxy

    idxval = np.zeros(tt * NI, np.int16)
    d1 = int(ngrp[BASE1:].sum())
    idxval[:d0] = np.repeat(
        np.arange(BASE1, dtype=np.int64), ngrp[:BASE1]
    ).astype(np.int16)
    idxval[t0 * NI : t0 * NI + d1] = np.repeat(
        np.arange(S_TOT - BASE1, dtype=np.int64), ngrp[BASE1:]
    ).astype(np.int16)
    # folded+replicated per gather unit: icd[16g+R, col] = idxval[col*16+R]
    units = unit_list(t0, t1)

    def fold(vals):
        a = vals.reshape(-1, 16).T                     # [16, n/16]
        return np.broadcast_to(a, (8, 16, a.shape[1])).reshape(P, -1)

    icd = np.zeros((len(units), P, 16 * M), np.int16)
    for u, (tb, nt_) in enumerate(units):
        icd[u, :, : nt_ * 8 * M] = fold(idxval[tb * NI : (tb + nt_) * NI])
    return (
        np.ascontiguousarray(wxy_flat.reshape(-1)),
        np.ascontiguousarray(icd.reshape(-1)),
        q,
    )


def _prepare(x, color, grid):
    """Full host prep: returns (nc, in_maps, qs, tt)."""
    b = x.shape[0]
    p3 = _build_p3(color[0], grid[0])
    per_core = [_prep_core(np.asarray(x[i], np.float32)) for i in range(b)]
    t0 = max(2, max((d0 + NI - 1) // NI for d0, _, _ in per_core))
    t1 = max((d1 + NI - 1) // NI for _, d1, _ in per_core)
    nc = _get_nc(t0, t1)
    in_maps = []
    qs = []
    for d0, d1, pdata in per_core:
        wxy_flat, icd, q = _build_arrays(pdata, t0, t1)
        in_maps.append({"p3": p3, "wxy": wxy_flat, "icd": icd})
        qs.append(q)
    return nc, in_maps, qs, t0 + t1


def kernel(x, color, grid):
    x = np.asarray(x, dtype=np.float32)
    color = np.asarray(color, dtype=np.float32)
    grid = np.asarray(grid, dtype=np.float32)
    b, n, _ = x.shape
    assert b == N_CORES and n == N_FULL
    nc, in_maps, qs, tt = _prepare(x, color, grid)
    res = run_bass_kernel_spmd(nc, in_maps, list(range(b)))
    out = np.empty((b, n, 6), np.float32)
    for i in range(b):
        flat = np.asarray(res.results[i]["out"]).reshape(tt * NT, 6)
        out[i] = flat[qs[i]].astype(np.float32)
    return out


# revision 41
# speedup vs baseline: 1.0026x; 1.0026x over previous
"""Trainium2 Bass kernel for nn_ColorGrid (bilinear grid_sample of two
[3,400,400] tables at 8x524288 points, sigmoid on the color channels).

Strategy (data-parallel over 8 NeuronCores, one batch row each):

  A naive kernel gathers one 256B brick per point with bulk dma_gather;
  at the cost-model floor of 22.76ns per 256B descriptor / 16 DMA
  engines that serializes to ~745us/core on the DMA engines.  Points
  are uniform, so on average ~13 points land in the same 3x3-cell
  brick: host-side grouping lets ONE descriptor serve G=6 points.

  1. Brick table P3 (built on host, f16): slot (a,b), a,b in [0,200],
     holds the 3x3 cell neighborhood rows 2a-1..2a+1, cols 2b-1..2b+1
     of both tables (zeros off-table), laid out [h:6][t:9] (t = 3*or+oc)
     in a 256B slot. A point with cell (y0,x0) needs the 2x2 corner
     window inside slot (a,b) = ((y0+1)//2, (x0+1)//2).
  2. Host computes per-point bilinear weights, scatters them into a
     9-tap f16 vector wxy aligned with the brick layout (4 nonzeros),
     sorts points by slot id and packs G=6 points per gather descriptor
     (padding partial groups with zero-weight dummies). Slot ids are
     binned in two row-ranges so bin-relative ids fit in int16; gather
     instructions for bin-1 tiles use a base-offset view of P3.
  3. Device per tile (C=126 point cols, M=21 groups/partition-row;
     gathers issued per tile-pair to halve SWDGE overhead, except the
     first two tiles which gather singly so compute starts sooner):
     load wxy + prefolded idx, dma_gather, multiply the gathered block
     (broadcast 0-stride over the G dim) by wxy on DVE, reduce the 9
     taps with an add tree split across DVE and the Pool engine,
     sigmoid the color channels in-place on ACT, store [128, C*6] f16.
     Engine balance per tile: DVE ~6.0us (products + tree top), Pool
     ~6.0us (descriptor gen + tree bottom), DMA ~5.3us, ACT ~0.5us.
  4. Host scatters the padded device output back to original point
     order and casts f32.
"""

import numpy as np

import concourse.bacc as bacc
import concourse.mybir as mybir
import concourse.tile as tile
from concourse import library_config
from concourse.bass_utils import run_bass_kernel_spmd
from concourse.tile_rust import add_dep_helper

F32 = mybir.dt.float32
F16 = mybir.dt.float16
I16 = mybir.dt.int16
Alu = mybir.AluOpType
Sigmoid = mybir.ActivationFunctionType.Sigmoid

P = 128            # SBUF partitions
W = 400            # table size
NA = 201           # brick rows (a = (y0+1)//2, y0 in [-1,399])
NB = 201           # brick cols
S_TOT = NA * NB    # 40401 slots
A_SPLIT = 101      # bin0: a in [0,100]; bin1: a in [101,200]
BASE1 = A_SPLIT * NB          # 20301 (slot-id offset of bin 1)
ELEM = 128         # f16 per slot (54 used)

G = 6              # points per gather descriptor (grouped by slot)
C = 168            # point cols per tile (must be divisible by G)
M = C // G         # gather blocks per partition-row
NT = P * C         # points per tile
NI = P * M         # gather idxs per tile
C_DVE = 98         # s2 column split: [0,C_DVE) on DVE, rest on Pool


def unit_list(t0, t1):
    """Gather units (tbase, ntile): tiles 0,1 single for fast start,
    then pairs within each bin, leftover bin tiles as singles."""
    assert t0 >= 2
    units = [(0, 1), (1, 1)]
    for lo, hi in ((2, t0), (t0, t0 + t1)):
        t = lo
        while t + 1 < hi:
            units.append((t, 2))
            t += 2
        if t < hi:
            units.append((t, 1))
    return units

N_CORES = 8
N_FULL = 524288


def build_nc(t0, t1, n_cores=N_CORES):
    """Compile the SPMD program: t0 bin-0 tiles followed by t1 bin-1 tiles.

    Gathers and input loads are issued per gather unit (see unit_list):
    mostly tile-pairs, halving the fixed SWDGE descriptor-generation
    overhead on Pool; single-tile units for fast pipeline fill and odd
    bin leftovers.
    """
    tt = t0 + t1
    units = unit_list(t0, t1)
    nc = bacc.Bacc(
        "TRN2", target_bir_lowering=False, debug=False, num_devices=n_cores
    )
    p3_d = nc.dram_tensor("p3", [S_TOT * ELEM], F16, kind="ExternalInput")
    w_d = nc.dram_tensor("wxy", [tt * NT * 9], F16, kind="ExternalInput")
    icd_d = nc.dram_tensor("icd", [len(units) * P * 16 * M], I16,
                           kind="ExternalInput")
    out_d = nc.dram_tensor("out", [tt * NT * 6], F16, kind="ExternalOutput")

    nu = len(units)
    p3_slots = p3_d.ap().rearrange("(s e) -> s e", e=ELEM)
    w_rows = w_d.ap().rearrange("(t p c n) -> t p (c n)", t=tt, p=P, n=9)
    icd_rows = icd_d.ap().rearrange("(u p m) -> u p m", u=nu, p=P)
    out_rows = out_d.ap().rearrange("(t p c h) -> t p (c h)", t=tt, p=P, h=6)

    with tile.TileContext(nc) as tc:
        lib = nc.gpsimd.load_library(library_config.mlp)

        with tc.tile_pool(name="inp", bufs=3) as pool_i, tc.tile_pool(
            name="work", bufs=4
        ) as pool_w, tc.tile_pool(name="outp", bufs=4) as pool_o:
            staged = {}

            def emit_gather(u):
                tbase, ntile = units[u]
                idx = pool_i.tile([P, 16 * M], I16, tag="idx")
                nc.sync.dma_start(
                    out=idx[:, : ntile * 8 * M], in_=icd_rows[u][:, : ntile * 8 * M]
                )
                g = pool_i.tile([P, 2 * M * ELEM], F16, tag="g")
                gv = g[:].rearrange("p (m e) -> p m e", e=ELEM)
                src = p3_slots if tbase < t0 else p3_slots[BASE1:]
                gi = nc.gpsimd.dma_gather(
                    gv[:, : ntile * M, :], src, idx[:, : ntile * 8 * M],
                    ntile * NI, ntile * NI, ELEM,
                    single_packet=False,
                )
                add_dep_helper(gi.ins, lib.ins, reason="gather needs mlp lib")
                wxy = pool_i.tile([P, 2 * C * 9], F16, tag="wxy")
                nc.sync.dma_start(
                    out=wxy[:, : ntile * C * 9],
                    in_=w_rows[tbase : tbase + ntile].transpose([1, 0, 2]),
                )
                for k in range(ntile):
                    staged[tbase + k] = (g, wxy, k)

            def emit_blend(t):
                last = t == tt - 1
                ve = nc.vector if last else nc.gpsimd
                cd = C if last else C_DVE
                g, wxy2, k = staged.pop(t)
                gv = g[:].rearrange("p (m e) -> p m e", e=ELEM)[
                    :, k * M : (k + 1) * M, :
                ]
                wv = wxy2[:].rearrange(
                    "p (k m g n) -> p k m g n", k=2, m=M, n=9
                )[:, k]

                # prod[p, c, h, t'] = g[p, c//G, 9h+t'] * wxy[p, c, t']
                pr = pool_w.tile([P, C * 54], F16, tag="pr")
                prv = pr[:].rearrange("p (c h n) -> p c h n", h=6, n=9)
                for h in range(6):
                    nc.vector.tensor_tensor(
                        prv[:, :, h, :].rearrange("p (m g) n -> p m g n", g=G),
                        gv[:, :, 9 * h : 9 * h + 9]
                        .unsqueeze(2)
                        .broadcast_to([P, M, G, 9]),
                        wv,
                        Alu.mult,
                    )
                # reduce the 9 taps: s1/s2 on DVE (s2 column-split with
                # Pool), s3 + tap-8 add on Pool
                s1 = pool_w.tile([P, C * 24], F16, tag="s1")
                s1v = s1[:].rearrange("p (c h n) -> p c h n", h=6, n=4)
                nc.vector.tensor_tensor(
                    s1v, prv[:, :, :, 0:4], prv[:, :, :, 4:8], Alu.add
                )
                s2 = pool_w.tile([P, C * 12], F16, tag="s2")
                s2v = s2[:].rearrange("p (c h n) -> p c h n", h=6, n=2)
                nc.vector.tensor_tensor(
                    s2v[:, :cd],
                    s1v[:, :cd, :, 0:2],
                    s1v[:, :cd, :, 2:4],
                    Alu.add,
                )
                if cd < C:
                    nc.gpsimd.tensor_tensor(
                        s2v[:, cd:],
                        s1v[:, cd:, :, 0:2],
                        s1v[:, cd:, :, 2:4],
                        Alu.add,
                    )
                s3 = pool_w.tile([P, C * 6], F16, tag="s3")
                s3v = s3[:].rearrange("p (c h) -> p c h", h=6)
                ve.tensor_tensor(
                    s3v, s2v[:, :, :, 0], s2v[:, :, :, 1], Alu.add
                )
                o6 = pool_o.tile([P, C * 6], F16, tag="o6")
                o6v = o6[:].rearrange("p (c h) -> p c h", h=6)
                ve.tensor_tensor(o6v, s3v, prv[:, :, :, 8], Alu.add)
                # sigmoid color channels in place, grid channels pass through
                nc.scalar.activation(o6v[:, :, 0:3], o6v[:, :, 0:3], Sigmoid)
                nc.scalar.dma_start(out=out_rows[t], in_=o6[:])

            tile_unit = {}
            for ui, (tb, nt_) in enumerate(units):
                for kk in range(nt_):
                    tile_unit[tb + kk] = ui
            emit_gather(0)
            emit_gather(1)
            emitted = 2
            for t in range(tt):
                # keep two units of lookahead beyond the one being consumed
                while emitted < min(tile_unit[t] + 3, nu):
                    emit_gather(emitted)
                    emitted += 1
                emit_blend(t)

    nc.compile()
    return nc


_NC_CACHE = {}


def _get_nc(t0, t1):
    key = (t0, t1)
    if key not in _NC_CACHE:
        _NC_CACHE[key] = build_nc(t0, t1)
    return _NC_CACHE[key]


def _build_p3(color, grid):
    """[S_TOT, ELEM] f16 brick table; slot (a,b) = [h:6][or:3][oc:3]."""
    tpad = np.zeros((6, W + 3, W + 3), np.float32)
    tpad[0:3, 1 : W + 1, 1 : W + 1] = color
    tpad[3:6, 1 : W + 1, 1 : W + 1] = grid
    p3 = np.zeros((NA, NB, ELEM), np.float16)
    for orr in range(3):
        for oc in range(3):
            v = tpad[:, orr : orr + 2 * NA : 2, oc : oc + 2 * NB : 2]
            for h in range(6):
                p3[:, :, 9 * h + 3 * orr + oc] = v[h].astype(np.float16)
    return p3.reshape(-1)


def _prep_core(x):
    """Per-core host prep.

    Returns (d0, d1, point_data) where point_data carries everything
    needed to build the device arrays once common tile counts are known.
    """
    n = x.shape[0]
    cx = x * np.float32(2.0) - np.float32(1.0)
    pos = ((cx + np.float32(1.0)) * np.float32(W) - np.float32(1.0)) * np.float32(0.5)
    f0 = np.floor(pos)
    w1 = pos - f0                       # [n, 2] f32: (wx1, wy1)
    k = f0.astype(np.int64) + 1         # [n, 2]: (kx, ky) in [0, 400]
    bcol = k[:, 0] >> 1
    oc0 = k[:, 0] & 1
    arow = k[:, 1] >> 1
    or0 = k[:, 1] & 1
    slot = arow * NB + bcol             # [n] int64 in [0, S_TOT)

    wx1 = w1[:, 0]
    wy1 = w1[:, 1]
    wx0 = np.float32(1.0) - wx1
    wy0 = np.float32(1.0) - wy1
    w4 = np.stack([wy0 * wx0, wy0 * wx1, wy1 * wx0, wy1 * wx1], axis=1)
    t00 = (or0 * 3 + oc0).astype(np.int64)
    wxy = np.zeros((n, 9), np.float16)
    cols = t00[:, None] + np.array([0, 1, 3, 4], np.int64)[None, :]
    np.put_along_axis(wxy, cols, w4.astype(np.float16), axis=1)

    counts = np.bincount(slot, minlength=S_TOT)
    ngrp = (counts + (G - 1)) // G
    gbase = np.concatenate([[0], np.cumsum(ngrp)[:-1]])
    starts = np.concatenate([[0], np.cumsum(counts)[:-1]])
    order = np.argsort(slot, kind="stable")
    rank = np.empty(n, np.int64)
    rank[order] = np.arange(n) - starts[slot[order]]

    d0 = int(ngrp[:BASE1].sum())
    d1 = int(ngrp[BASE1:].sum())
    return d0, d1, (slot, rank, gbase, ngrp, wxy, d0)


def _build_arrays(point_data, t0, t1):
    """Device arrays for one core at common tile counts (t0, t1)."""
    slot, rank, gbase, ngrp, wxy, d0 = point_data
    tt = t0 + t1
    n = slot.shape[0]

    # group sequence position: bin0 groups at [0, d0); bin1 at t0*NI + ...
    seqbase = gbase.copy()
    seqbase[BASE1:] += t0 * NI - d0
    seq = seqbase[slot] + rank // G
    j = rank % G
    ti = seq // NI
    i = seq % NI
    q = (ti * P + (i % P)) * C + (i // P) * G + j   # padded flat position

    wxy_flat = np.zeros((tt * NT, 9), np.float16)
    wxy_flat[q] = wxy

    idxval = np.zeros(tt * NI, np.int16)
    d1 = int(ngrp[BASE1:].sum())
    idxval[:d0] = np.repeat(
        np.arange(BASE1, dtype=np.int64), ngrp[:BASE1]
    ).astype(np.int16)
    idxval[t0 * NI : t0 * NI + d1] = np.repeat(
        np.arange(S_TOT - BASE1, dtype=np.int64), ngrp[BASE1:]
    ).astype(np.int16)
    # folded+replicated per gather unit: icd[16g+R, col] = idxval[col*16+R]
    units = unit_list(t0, t1)

    def fold(vals):
        a = vals.reshape(-1, 16).T                     # [16, n/16]
        return np.broadcast_to(a, (8, 16, a.shape[1])).reshape(P, -1)

    icd = np.zeros((len(units), P, 16 * M), np.int16)
    for u, (tb, nt_) in enumerate(units):
        icd[u, :, : nt_ * 8 * M] = fold(idxval[tb * NI : (tb + nt_) * NI])
    return (
        np.ascontiguousarray(wxy_flat.reshape(-1)),
        np.ascontiguousarray(icd.reshape(-1)),
        q,
    )


def _prepare(x, color, grid):
    """Full host prep: returns (nc, in_maps, qs, tt)."""
    b = x.shape[0]
    p3 = _build_p3(color[0], grid[0])
    per_core = [_prep_core(np.asarray(x[i], np.float32)) for i in range(b)]
    t0 = max(2, max((d0 + NI - 1) // NI for d0, _, _ in per_core))
    t1 = max((d1 + NI - 1) // NI for _, d1, _ in per_core)
    nc = _get_nc(t0, t1)
    in_maps = []
    qs = []
    for d0, d1, pdata in per_core:
        wxy_flat, icd, q = _build_arrays(pdata, t0, t1)
        in_maps.append({"p3": p3, "wxy": wxy_flat, "icd": icd})
        qs.append(q)
    return nc, in_maps, qs, t0 + t1


def kernel(x, color, grid):
    x = np.asarray(x, dtype=np.float32)
    color = np.asarray(color, dtype=np.float32)
    grid = np.asarray(grid, dtype=np.float32)
    b, n, _ = x.shape
    assert b == N_CORES and n == N_FULL
    nc, in_maps, qs, tt = _prepare(x, color, grid)
    res = run_bass_kernel_spmd(nc, in_maps, list(range(b)))
    out = np.empty((b, n, 6), np.float32)
    for i in range(b):
        flat = np.asarray(res.results[i]["out"]).reshape(tt * NT, 6)
        out[i] = flat[qs[i]].astype(np.float32)
    return out
